# revision 12
# baseline (speedup 1.0000x reference)
"""DANet attention (PAM + CAM + fuse conv3x3 + BN + ReLU) on 8 TRN2 NeuronCores.

Sharding: core = 2*b + j handles sample b (of 4) and spatial band j (of 2).
Each band covers 34 rows of the 64-row image (32 output rows + 1 halo row on
each side; out-of-image halo rows are zero). PAM/CAM are computed for the
band's 2176 positions against the full 4096-position sample; the 3x3 fuse
conv runs on a zero-padded 34x66 layout; BN batch stats are combined across
all 8 cores with a single tiny AllReduce.

Perf structure (v2):
- chunked input DMAs so compute starts early
- k/q projections emit 4 partition-replicas (host-replicated weights) so the
  K=32 PAM energy matmuls can be row-tiled two-at-a-time into one 2-bank
  PSUM tile; exp processes [128, 2*ln] per pair on the scalar engine
- exp results staged in SBUF per quarter-tile (8 m-chunks), PAM apply
  matmuls run as dense 8-MM accumulation bursts that pipeline against the
  next quarter's energy/exp work
- biases/gammas folded: q/k bias via DVE tensor_scalar_add on the PSUM
  evacuation; gamma_p via 1/gp in the softmax denominator reduction and a
  host-side gp*v_b column in the final combine; gamma_c into attn_c
- BN stats for both channel chunks combined into one AllReduce

Self-contained: hardcodes shapes B=4, C=256, H=W=64, RC=32.
"""

import numpy as np
import ml_dtypes

import concourse.bass as bass
import concourse.tile as tile
from concourse import bacc, mybir
from concourse.bass_utils import run_bass_kernel_spmd
F32 = mybir.dt.float32
F32R = mybir.dt.float32r
BF16 = mybir.dt.bfloat16
AF = mybir.ActivationFunctionType
ALU = mybir.AluOpType

B, C, H, W = 4, 256, 64, 64
N = H * W            # 4096
RC = 32              # reduced channels for q/k
NB = 34 * W          # 2176 band positions (34 rows incl. halo/phantom rows)
PADW = W + 2         # 66
FLAT = 34 * PADW     # 2244 padded band slots
CCH = 2              # channel chunks of 128 (C = 256)
ICH = 4              # conv input-channel chunks of 128 (2C = 512)
MG = N // 128        # 32 m-chunks in PAM contraction
BN_EPS = 1e-5

# PAM/CAM band tiles: (start, len) over the 2176 band positions
PAM_TILES = [(0, 512), (512, 512), (1024, 512), (1536, 512), (2048, 128)]
# conv output tiles: (slot_start, len) over padded flat coords; valid output
# rows are padded rows 1..32 -> flat [66, 2178)
CONV_TILES = [(66, 512), (578, 512), (1090, 512), (1602, 512), (2114, 64)]
# stats chunks over the 2112-long y strip
ST_CHUNKS = [(0, 512), (512, 512), (1024, 512), (1536, 512), (2048, 64)]


def build(n_cores=8, stat_count=4 * N):
    """Build and compile the SPMD kernel graph. Returns compiled Bacc."""
    nc = bacc.Bacc("TRN2", target_bir_lowering=False, debug=False,
                   num_devices=n_cores)

    # ---- DRAM parameters (per core) ----
    x_full = nc.dram_tensor("x_full", [C, N], F32R, kind="ExternalInput")
    x_band = nc.dram_tensor("x_band", [C, NB], F32R, kind="ExternalInput")
    qwT4_d = nc.dram_tensor("qwT4", [C, 128], F32R, kind="ExternalInput")
    kwT4_d = nc.dram_tensor("kwT4", [C, 128], F32R, kind="ExternalInput")
    vwT_d = nc.dram_tensor("vwT", [C, C], BF16, kind="ExternalInput")
    qb4_d = nc.dram_tensor("qb4", [128, 1], F32, kind="ExternalInput")
    kb4_d = nc.dram_tensor("kb4", [128, 1], F32, kind="ExternalInput")
    fw_d = nc.dram_tensor("fw", [ICH, 128, 9 * 2 * 128], BF16, kind="ExternalInput")
    gpinv_d = nc.dram_tensor("gpinv", [128, 1], BF16, kind="ExternalInput")
    gpvb_d = nc.dram_tensor("gpvb", [128, CCH], F32, kind="ExternalInput")
    gc_d = nc.dram_tensor("gc", [128, 1], F32, kind="ExternalInput")
    mf_d = nc.dram_tensor("mf", [128, 1], F32, kind="ExternalInput")   # first-row mask
    ml_d = nc.dram_tensor("ml", [128, 1], F32, kind="ExternalInput")   # last-row mask
    bnsc_d = nc.dram_tensor("bnsc", [128, CCH], F32, kind="ExternalInput")
    bnbi_d = nc.dram_tensor("bnbi", [128, CCH], F32, kind="ExternalInput")
    ones_m1b_d = nc.dram_tensor("ones_m1b", [1, 128], BF16, kind="ExternalInput")
    ident_d = nc.dram_tensor("ident", [128, 128], F32R, kind="ExternalInput")
    y_out = nc.dram_tensor("y_out", [CCH, 128, 32 * W], F32, kind="ExternalOutput")
    st_loc = nc.dram_tensor("st_loc", [128, 4], F32)
    st_glob = nc.dram_tensor("st_glob", [128, 4], F32, addr_space="Shared")

    with tile.TileContext(nc) as tc:
        with tc.tile_pool(name="persist", bufs=1) as pp, \
             tc.tile_pool(name="psum", bufs=1, space="PSUM") as psp:

            # ---- persistent SBUF tiles ----
            k_sb4 = pp.tile([128, N], BF16)        # k, 4 partition-replicas
            q_sb4 = pp.tile([128, NB], BF16)       # q, 4 partition-replicas
            vT = pp.tile([128, MG * C], BF16)      # v^T, m on partitions
            xb = [pp.tile([128, NB], F32R, name=f"xb{i}") for i in range(CCH)]
            qwT4 = pp.tile([128, CCH * 128], F32R)
            kwT4 = pp.tile([128, CCH * 128], F32R)
            vwT = pp.tile([128, CCH * C], BF16)
            qb4 = pp.tile([128, 1], F32)
            kb4 = pp.tile([128, 1], F32)
            gpinv = pp.tile([128, 1], BF16)
            gpvb = pp.tile([128, CCH], F32)
            gc = pp.tile([128, 1], F32)
            mf = pp.tile([128, 1], F32)
            ml = pp.tile([128, 1], F32)
            bnsc = pp.tile([128, CCH], F32)
            bnbi = pp.tile([128, CCH], F32)
            ones_m1b = pp.tile([1, 128], BF16)
            ident = pp.tile([128, 128], F32R)
            fw = [pp.tile([128, 9 * 2 * 128], BF16, name=f"fw{i}") for i in range(ICH)]
            feats = [pp.tile([128, FLAT + 2], BF16, name=f"feats{i}") for i in range(ICH)]
            feats_sh = [pp.tile([128, FLAT + 2], BF16, name=f"feats_sh{i}")
                        for i in range(ICH)]
            y_sb = [pp.tile([128, 32 * PADW], F32, name=f"y_sb{i}") for i in range(CCH)]
            st_sb = pp.tile([128, 4], F32)
            stg = pp.tile([128, 4], F32)
            epsc = pp.tile([128, 1], F32)

            # preload the exp table set while DMAs run
            nc.gpsimd.memset(epsc, BN_EPS)
            dummy = pp.tile([128, 1], F32)
            nc.scalar.activation(dummy, epsc, AF.Exp)

            # ---- chunked input loads: x_band then x_full, 512-col pieces ----
            for i in range(CCH):
                for (s, ln) in PAM_TILES:
                    nc.sync.dma_start(
                        out=xb[i][:, s:s + ln],
                        in_=x_band.ap()[i * 128:(i + 1) * 128, s:s + ln])
            nc.sync.dma_start(out=qwT4[:, 0:128], in_=qwT4_d.ap()[0:128, :])
            nc.sync.dma_start(out=qwT4[:, 128:256], in_=qwT4_d.ap()[128:256, :])
            nc.sync.dma_start(out=kwT4[:, 0:128], in_=kwT4_d.ap()[0:128, :])
            nc.sync.dma_start(out=kwT4[:, 128:256], in_=kwT4_d.ap()[128:256, :])
            nc.sync.dma_start(out=vwT[:, 0:C], in_=vwT_d.ap()[0:128, :])
            nc.sync.dma_start(out=vwT[:, C:2 * C], in_=vwT_d.ap()[128:256, :])
            for t, d in ((qb4, qb4_d), (kb4, kb4_d), (gpinv, gpinv_d),
                         (gpvb, gpvb_d), (gc, gc_d), (mf, mf_d), (ml, ml_d),
                         (bnsc, bnsc_d), (bnbi, bnbi_d),
                         (ones_m1b, ones_m1b_d), (ident, ident_d)):
                nc.sync.dma_start(out=t, in_=d.ap())
            for i in range(ICH):
                nc.sync.dma_start(out=fw[i], in_=fw_d.ap()[i])
            for i in range(ICH):
                nc.gpsimd.memset(feats[i], 0.0)

            # ================= phase 1: xf-derived tensors =================
            with tc.tile_pool(name="early", bufs=1) as ep:
                xf = [ep.tile([128, N], F32R, name=f"xf{i}") for i in range(CCH)]
                xf_b = [ep.tile([128, N], BF16, name=f"xf_b{i}") for i in range(CCH)]
                ecp = [psp.tile([128, C], F32, tag="u", bufs=2, name=f"ecp{i}")
                       for i in range(CCH)]

                for t in range(N // 512):
                    for i in range(CCH):
                        nc.sync.dma_start(
                            out=xf[i][:, t * 512:(t + 1) * 512],
                            in_=x_full.ap()[i * 128:(i + 1) * 128,
                                            t * 512:(t + 1) * 512])
                    for i in range(CCH):
                        nc.vector.tensor_copy(xf_b[i][:, t * 512:(t + 1) * 512],
                                              xf[i][:, t * 512:(t + 1) * 512])
                    # k projection (4 replicas on partitions), bias on the copy
                    kp = psp.tile([128, 512], F32, tag="wk", bufs=2, name="kp")
                    for ct in range(CCH):
                        nc.tensor.matmul(kp, lhsT=kwT4[:, ct * 128:(ct + 1) * 128],
                                         rhs=xf[ct][:, t * 512:(t + 1) * 512],
                                         start=(ct == 0), stop=(ct == CCH - 1))
                    nc.vector.tensor_scalar_add(k_sb4[:, t * 512:(t + 1) * 512],
                                                kp, kb4)
                    # q projection on the band
                    if t < len(PAM_TILES):
                        (sq_, ln) = PAM_TILES[t]
                        qp = psp.tile([128, 512], F32, tag="wk", bufs=2, name="qp")
                        for ct in range(CCH):
                            nc.tensor.matmul(qp[:, 0:ln],
                                             lhsT=qwT4[:, ct * 128:(ct + 1) * 128],
                                             rhs=xb[ct][:, sq_:sq_ + ln],
                                             start=(ct == 0), stop=(ct == CCH - 1))
                        nc.vector.tensor_scalar_add(q_sb4[:, sq_:sq_ + ln],
                                                    qp[:, 0:ln], qb4)
                    for g in range(4 * t, 4 * t + 4):
                        # vT chunk g
                        vp = psp.tile([128, C], F32, tag="wk", bufs=2, name="vp")
                        for ct in range(CCH):
                            nc.tensor.matmul(vp,
                                             lhsT=xf_b[ct][:, g * 128:(g + 1) * 128],
                                             rhs=vwT[:, ct * C:(ct + 1) * C],
                                             start=(ct == 0), stop=(ct == CCH - 1))
                        nc.scalar.activation(vT[:, g * C:(g + 1) * C], vp, AF.Copy)
                        # xfT chunk g + CAM energy accumulation
                        xfT = ep.tile([128, C], F32R, tag="xfT", bufs=3, name="xfT")
                        for ct in range(CCH):
                            tp = psp.tile([128, 128], F32R, tag="et", bufs=2,
                                          name="tp")
                            nc.tensor.transpose(tp, xf[ct][:, g * 128:(g + 1) * 128],
                                                ident)
                            nc.vector.tensor_copy(xfT[:, ct * 128:(ct + 1) * 128], tp)
                        for ct in range(CCH):
                            nc.tensor.matmul(ecp[ct],
                                             lhsT=xfT[:, ct * 128:(ct + 1) * 128],
                                             rhs=xfT, start=(g == 0),
                                             stop=(g == MG - 1))

                # ---- phase 2: CAM softmax + apply (gamma_c folded in) ----
                attn_cT = ep.tile([128, CCH * C], F32R)
                for ct in range(CCH):
                    emin = ep.tile([128, 1], F32, tag="cam_t", bufs=8, name="emin")
                    esum = ep.tile([128, 1], F32, tag="cam_t", bufs=8, name="esum")
                    erec = ep.tile([128, 1], F32, tag="cam_t", bufs=8, name="erec")
                    ea = ep.tile([128, C], F32, tag="ea", bufs=2, name="ea")
                    attn_c = ep.tile([128, C], F32R, tag="attn_c", bufs=2,
                                     name="attn_c")
                    nc.vector.tensor_reduce(emin, ecp[ct], axis=mybir.AxisListType.X,
                                            op=ALU.min)
                    nc.scalar.activation(ea, ecp[ct], AF.Exp, bias=emin, scale=-1.0,
                                         accum_out=esum)
                    nc.vector.reciprocal(erec, esum)
                    # attn_c = (ea * erec) * gamma_c
                    nc.vector.tensor_scalar(attn_c, ea, erec, gc,
                                            op0=ALU.mult, op1=ALU.mult)
                    for dc in range(CCH):
                        tp2 = psp.tile([128, 128], F32R, tag="et", bufs=2, name="tp2")
                        nc.tensor.transpose(tp2, attn_c[:, dc * 128:(dc + 1) * 128],
                                            ident)
                        nc.vector.tensor_copy(
                            attn_cT[:, (dc * CCH + ct) * 128:
                                    (dc * CCH + ct + 1) * 128], tp2)

                # cam_out = attn_c @ x_band; channel = cam + x_band -> feats[2..3]
                for ct in range(CCH):
                    for (s, ln) in PAM_TILES:
                        cp = psp.tile([128, 512], F32, tag="wk", bufs=2, name="cp")
                        for dc in range(CCH):
                            nc.tensor.matmul(
                                cp[:, 0:ln],
                                lhsT=attn_cT[:, (dc * CCH + ct) * 128:
                                             (dc * CCH + ct + 1) * 128],
                                rhs=xb[dc][:, s:s + ln],
                                start=(dc == 0), stop=(dc == CCH - 1))
                        r0, nr = s // W, ln // W
                        fdst = feats[CCH + ct][:, 2:2 + 34 * PADW] \
                            .rearrange("p (r w) -> p r w", w=PADW)[:, r0:r0 + nr, 0:W]
                        fsrc_cam = cp[:, 0:ln].rearrange("p (r w) -> p r w", w=W)
                        fx = xb[ct][:, s:s + ln].rearrange("p (r w) -> p r w", w=W)
                        nc.vector.tensor_add(fdst, fsrc_cam, fx)

            # ============ phases 3-5 use the late pool (reuses early space) ====
            with tc.tile_pool(name="late", bufs=1) as lp:
                # ---- phase 3: PAM pipeline ----
                def pam_tail(ti, s, ln, u, s_acc):
                    # softmax denominator: rb = broadcast(gp / s)
                    s512 = lp.tile([128, 512], BF16, tag="s512", bufs=2, name="s512")
                    nc.vector.tensor_add(s512[:, 0:ln], s_acc[:, 0:ln],
                                         s_acc[:, ln:2 * ln])
                    ssum = psp.tile([1, 512], F32, tag="wk", bufs=2, name="ssum")
                    nc.tensor.matmul(ssum[:, 0:ln], lhsT=gpinv, rhs=s512[:, 0:ln],
                                     start=True, stop=True)
                    ssb = lp.tile([1, 512], BF16, tag="ssb", bufs=2, name="ssb")
                    nc.vector.tensor_copy(ssb[:, 0:ln], ssum[:, 0:ln])
                    rb = psp.tile([128, 512], F32, tag="wk", bufs=2, name="rb")
                    nc.tensor.matmul(rb[:, 0:ln], lhsT=ones_m1b, rhs=ssb[:, 0:ln],
                                     start=True, stop=True)
                    rb_sb = lp.tile([128, 512], F32, tag="rb_sb", bufs=2,
                                    name="rb_sb")
                    nc.vector.reciprocal_approx_fast(rb_sb[:, 0:ln], rb[:, 0:ln])
                    # position = (u * gp/s + gp*vb) + x_band -> feats[0..1]
                    r0, nr = s // W, ln // W
                    for ct in range(CCH):
                        tmp = lp.tile([128, 512], F32, tag="tmp", bufs=2, name="tmp")
                        nc.vector.tensor_mul(tmp[:, 0:ln], u[ct][:, 0:ln],
                                             rb_sb[:, 0:ln])
                        fdst = feats[ct][:, 2:2 + 34 * PADW] \
                            .rearrange("p (r w) -> p r w", w=PADW)[:, r0:r0 + nr, 0:W]
                        fsrc = tmp[:, 0:ln].rearrange("p (r w) -> p r w", w=W)
                        fx = xb[ct][:, s:s + ln].rearrange("p (r w) -> p r w", w=W)
                        nc.vector.scalar_tensor_tensor(fdst, fsrc,
                                                       gpvb[:, ct:ct + 1], fx,
                                                       op0=ALU.add, op1=ALU.add)
                    # phantom halo rows: row 0 after first tile, row 33 after last
                    if ti == 0:
                        for i in range(ICH):
                            fv = feats[i][:, 2:2 + 34 * PADW] \
                                .rearrange("p (r w) -> p r w", w=PADW)
                            nc.vector.tensor_scalar_mul(fv[:, 0:1, 0:W],
                                                        fv[:, 0:1, 0:W], mf)
                    if ti == len(PAM_TILES) - 1:
                        for i in range(ICH):
                            fv = feats[i][:, 2:2 + 34 * PADW] \
                                .rearrange("p (r w) -> p r w", w=PADW)
                            nc.vector.tensor_scalar_mul(fv[:, 33:34, 0:W],
                                                        fv[:, 33:34, 0:W], ml)

                pend = None
                for ti, (s, ln) in enumerate(PAM_TILES):
                    u = [psp.tile([128, 512], F32, tag="u", bufs=2, name=f"u{i}")
                         for i in range(CCH)]
                    s_acc = lp.tile([128, 1024], BF16, tag="s_acc", bufs=2,
                                    name="s_acc")
                    for qt in range(4):      # quarter-tiles of 8 m-chunks
                        esb = lp.tile([128, 8 * 512], BF16, tag="esb", bufs=3,
                                      name="esb")
                        for p in range(4):   # pairs of m-chunks
                            g0 = qt * 8 + p * 2
                            rg = (0, 32) if p % 2 == 0 else (64, 96)
                            et = psp.tile([128, 1024], F32, tag="et", bufs=2,
                                          name="et")
                            for b in range(2):
                                gb = g0 + b
                                nc.tensor.matmul(
                                    et[:, b * 512:b * 512 + ln],
                                    lhsT=k_sb4[rg[b]:rg[b] + 32,
                                               gb * 128:(gb + 1) * 128],
                                    rhs=q_sb4[rg[b]:rg[b] + 32, s:s + ln],
                                    start=True, stop=True,
                                    tile_position=(rg[b], 0))
                            eo = p * 2 * ln
                            if ln == 512:
                                nc.scalar.activation(esb[:, eo:eo + 1024], et,
                                                     AF.Exp)
                            else:
                                ev = et.rearrange("p (b x) -> p b x", b=2)[:, :, 0:ln]
                                nc.scalar.activation(
                                    esb[:, eo:eo + 2 * ln]
                                    .rearrange("p (b x) -> p b x", b=2), ev, AF.Exp)
                            # denominator partial sums on DVE
                            if qt == 0 and p == 0:
                                nc.vector.tensor_copy(s_acc[:, 0:2 * ln],
                                                      esb[:, eo:eo + 2 * ln])
                            else:
                                nc.vector.tensor_add(s_acc[:, 0:2 * ln],
                                                     s_acc[:, 0:2 * ln],
                                                     esb[:, eo:eo + 2 * ln])
                        # dense apply runs for this quarter
                        for ct in range(CCH):
                            for gl in range(8):
                                g = qt * 8 + gl
                                nc.tensor.matmul(
                                    u[ct][:, 0:ln],
                                    lhsT=vT[:, g * C + ct * 128:
                                            g * C + (ct + 1) * 128],
                                    rhs=esb[:, gl * ln:(gl + 1) * ln],
                                    start=(g == 0), stop=(g == MG - 1))
                        # previous tile's combine, emitted one quarter late so
                        # this tile's energy matmuls precede it in PE priority
                        if qt == 0 and pend is not None:
                            pam_tail(*pend)
                    pend = (ti, s, ln, u, s_acc)
                pam_tail(*pend)

                # shifted copy so odd-offset conv taps read 4-byte-aligned
                for i in range(ICH):
                    nc.vector.tensor_copy(feats_sh[i][:, 0:FLAT + 1],
                                          feats[i][:, 1:FLAT + 2])

                # ---- phase 4+5: conv3x3 with fused BN stats, per-ot
                # AllReduce so chunk 0's BN/relu/store hides under chunk 1 ----
                inv_n = 1.0 / float(stat_count)
                for ot in range(CCH):
                    sparts = [lp.tile([128, len(CONV_TILES)], F32, tag="sparts",
                                      bufs=4, name=f"sparts{i}") for i in range(2)]
                    for ci, (s, ln) in enumerate(CONV_TILES):
                        yp = psp.tile([128, 512], F32, tag="u", bufs=2, name="yp")
                        first = True
                        for tap in range(9):
                            off = (tap // 3 - 1) * PADW + (tap % 3 - 1)
                            o = 1 + s + off
                            for ic in range(ICH):
                                src = feats[ic][:, o:o + ln] if o % 2 == 0 \
                                    else feats_sh[ic][:, o - 1:o - 1 + ln]
                                nc.tensor.matmul(
                                    yp[:, 0:ln],
                                    lhsT=fw[ic][:, (tap * 2 + ot) * 128:
                                                (tap * 2 + ot + 1) * 128],
                                    rhs=src,
                                    start=first, stop=(tap == 8 and ic == ICH - 1))
                                first = False
                        a = s - 66
                        nc.vector.tensor_copy(y_sb[ot][:, a:a + ln], yp[:, 0:ln])
                        # zero the junk pad cols inside this strip, then stats
                        yv = y_sb[ot].rearrange("p (r w) -> p r w", w=PADW)
                        for c in (0, 65):
                            r0 = max(0, (a - c + PADW - 1) // PADW)
                            r1 = min(32, (a + ln - 1 - c) // PADW + 1)
                            if r1 > r0:
                                nc.vector.memset(yv[:, r0:r1, c:c + 1], 0.0)
                        nc.vector.tensor_reduce(sparts[0][:, ci:ci + 1],
                                                y_sb[ot][:, a:a + ln],
                                                axis=mybir.AxisListType.X, op=ALU.add)
                        sq = lp.tile([128, 512], F32, tag="sq", bufs=2, name="sq")
                        nc.scalar.activation(sq[:, 0:ln], y_sb[ot][:, a:a + ln],
                                             AF.Square,
                                             accum_out=sparts[1][:, ci:ci + 1])
                    for i in range(2):
                        nc.vector.tensor_reduce(st_sb[:, 2 * ot + i:2 * ot + i + 1],
                                                sparts[i],
                                                axis=mybir.AxisListType.X, op=ALU.add)

                # ---- phase 5: one AllReduce, BN math, relu, store ----
                nc.sync.dma_start(out=st_loc.ap(), in_=st_sb)
                if n_cores > 1:
                    nc.gpsimd.collective_compute(
                        "AllReduce", ALU.add,
                        replica_groups=[list(range(n_cores))],
                        ins=[st_loc.ap()], outs=[st_glob.ap()])
                else:
                    nc.gpsimd.dma_start(out=st_glob.ap(), in_=st_loc.ap())
                nc.sync.dma_start(out=stg, in_=st_glob.ap())

                # BN math for both channel chunks at once ([128, 2] columns)
                mean = lp.tile([128, 2], F32, tag="bn_t", bufs=8, name="mean")
                msq = lp.tile([128, 2], F32, tag="bn_t", bufs=8, name="msq")
                var = lp.tile([128, 2], F32, tag="bn_t", bufs=8, name="var")
                m2 = lp.tile([128, 2], F32, tag="bn_t", bufs=8, name="m2")
                std = lp.tile([128, 2], F32, tag="bn_t", bufs=8, name="std")
                rstd = lp.tile([128, 2], F32, tag="bn_t", bufs=8, name="rstd")
                sc2 = lp.tile([128, 2], F32, tag="bn_t", bufs=8, name="sc2")
                bi2 = lp.tile([128, 2], F32, tag="bn_t", bufs=8, name="bi2")
                t0 = lp.tile([128, 2], F32, tag="bn_t", bufs=8, name="t0")
                stv = stg.rearrange("p (o i) -> p o i", i=2)
                nc.vector.tensor_scalar_mul(mean, stv[:, :, 0], inv_n)
                nc.vector.tensor_scalar_mul(msq, stv[:, :, 1], inv_n)
                nc.vector.tensor_mul(m2, mean, mean)
                nc.vector.tensor_sub(var, msq, m2)
                nc.scalar.activation(std, var, AF.Sqrt, bias=epsc)
                nc.vector.reciprocal(rstd, std)
                nc.vector.tensor_mul(sc2, bnsc, rstd)
                nc.vector.tensor_mul(t0, mean, sc2)
                nc.vector.tensor_sub(bi2, bnbi, t0)
                # y = relu(y*scale' + bias') on valid cols, then store
                for ot in range(CCH):
                    stage = lp.tile([128, 32 * W], F32, tag="stage", bufs=4,
                                    name="stage")
                    yvv = y_sb[ot].rearrange("p (r w) -> p r w", w=PADW)
                    for hf in range(2):
                        ysrc = yvv[:, hf * 16:hf * 16 + 16, 1:65]
                        nc.scalar.activation(
                            stage[:, hf * 1024:(hf + 1) * 1024]
                            .rearrange("p (r w) -> p r w", w=W), ysrc, AF.Relu,
                            bias=bi2[:, ot:ot + 1], scale=sc2[:, ot:ot + 1])
                        for dq in range(2):
                            o = hf * 1024 + dq * 512
                            nc.sync.dma_start(
                                out=y_out.ap()[ot][:, o:o + 512],
                                in_=stage[:, o:o + 512])

    nc.compile()
    return nc


_CACHE = {}


def _get_nc(n_cores=8, stat_count=4 * N):
    key = (n_cores, stat_count)
    if key not in _CACHE:
        _CACHE[key] = build(n_cores, stat_count)
    return _CACHE[key]


def make_in_maps(x, q_w, q_b, k_w, k_b, v_w, v_b, gamma_p, gamma_c,
                 fuse_w, bn_scale, bn_bias, cores=8):
    f4 = np.float32
    shared = {}
    # 4 partition-replicas of the q/k projection weights and biases
    qwT = np.asarray(q_w, f4).T                  # [C, RC]
    kwT = np.asarray(k_w, f4).T
    shared["qwT4"] = np.ascontiguousarray(np.tile(qwT, (1, 4)))   # [C, 128]
    shared["kwT4"] = np.ascontiguousarray(np.tile(kwT, (1, 4)))
    shared["qb4"] = np.tile(np.asarray(q_b, f4), 4).reshape(128, 1)
    shared["kb4"] = np.tile(np.asarray(k_b, f4), 4).reshape(128, 1)
    shared["vwT"] = np.ascontiguousarray(np.asarray(v_w, f4).T).astype(ml_dtypes.bfloat16)
    # fuse_w [256, 512, 3, 3] -> [ic, i, tap*2*128 + ot*128 + o] in bf16
    fwr = np.asarray(fuse_w, f4).reshape(CCH, 128, ICH, 128, 3, 3)
    fwt = np.ascontiguousarray(fwr.transpose(2, 3, 4, 5, 0, 1))  # ic,i,kh,kw,ot,o
    shared["fw"] = fwt.reshape(ICH, 128, 9 * 2 * 128).astype(ml_dtypes.bfloat16)
    gp = np.asarray(gamma_p, f4).ravel()[0]
    gcv = np.asarray(gamma_c, f4).ravel()[0]
    with np.errstate(divide="ignore"):
        gpi = np.float32(1.0) / gp
    shared["gpinv"] = np.full((128, 1), gpi, f4).astype(ml_dtypes.bfloat16)
    shared["gpvb"] = np.ascontiguousarray(
        (gp * np.asarray(v_b, f4)).reshape(CCH, 128).T)
    shared["gc"] = np.full((128, 1), gcv, f4)
    shared["bnsc"] = np.ascontiguousarray(np.asarray(bn_scale, f4).reshape(CCH, 128).T)
    shared["bnbi"] = np.ascontiguousarray(np.asarray(bn_bias, f4).reshape(CCH, 128).T)
    shared["ones_m1b"] = np.ones((1, 128), ml_dtypes.bfloat16)
    shared["ident"] = np.eye(128, dtype=f4)

    xs = np.asarray(x, f4).reshape(B, C, N)
    zrow = np.zeros((C, W), f4)
    in_maps = []
    for core in range(cores):
        b, j = core // 2, core % 2
        xf = np.ascontiguousarray(xs[b])
        if j == 0:
            band = np.concatenate([zrow, xf[:, 0:33 * W]], axis=1)
            mfv, mlv = 0.0, 1.0
        else:
            band = np.concatenate([xf[:, 31 * W:], zrow], axis=1)
            mfv, mlv = 1.0, 0.0
        m = dict(shared)
        m["x_full"] = xf
        m["x_band"] = np.ascontiguousarray(band)
        m["mf"] = np.full((128, 1), mfv, f4)
        m["ml"] = np.full((128, 1), mlv, f4)
        in_maps.append(m)
    return in_maps


def kernel(**inputs):
    nc = _get_nc(8)
    in_maps = make_in_maps(**inputs)
    r = run_bass_kernel_spmd(nc, in_maps, core_ids=list(range(8)))
    out = np.empty((B, C, H, W), np.float32)
    for core in range(8):
        b, j = core // 2, core % 2
        y2 = r.results[core]["y_out"]  # [2, 128, 2048]
        out[b, 0:128, 32 * j:32 * j + 32, :] = y2[0].reshape(128, 32, W)
        out[b, 128:256, 32 * j:32 * j + 32, :] = y2[1].reshape(128, 32, W)
    return out


# revision 24
# speedup vs baseline: 1.0837x; 1.0837x over previous
"""DANet attention (PAM + CAM + fuse conv3x3 + BN + ReLU) on 8 TRN2 NeuronCores.

Sharding: core = 2*b + j handles sample b (of 4) and spatial band j (of 2).
Each band covers 34 rows of the 64-row image (32 output rows + 1 halo row on
each side; out-of-image halo rows are zero). PAM/CAM are computed for the
band's 2176 positions against the full 4096-position sample; the 3x3 fuse
conv runs on a zero-padded 34x66 layout; BN batch stats are combined across
all 8 cores with a single tiny AllReduce.

Perf structure (v2):
- chunked input DMAs so compute starts early
- k/q projections emit 4 partition-replicas (host-replicated weights) so the
  K=32 PAM energy matmuls can be row-tiled two-at-a-time into one 2-bank
  PSUM tile; exp processes [128, 2*ln] per pair on the scalar engine
- exp results staged in SBUF per quarter-tile (8 m-chunks), PAM apply
  matmuls run as dense 8-MM accumulation bursts that pipeline against the
  next quarter's energy/exp work
- biases/gammas folded: q/k bias via DVE tensor_scalar_add on the PSUM
  evacuation; gamma_p via 1/gp in the softmax denominator reduction and a
  host-side gp*v_b column in the final combine; gamma_c into attn_c
- BN stats for both channel chunks combined into one AllReduce

Self-contained: hardcodes shapes B=4, C=256, H=W=64, RC=32.
"""

import numpy as np
import ml_dtypes

import concourse.bass as bass
import concourse.tile as tile
from concourse import bacc, mybir
from concourse.bass_utils import run_bass_kernel_spmd
F32 = mybir.dt.float32
F32R = mybir.dt.float32r
BF16 = mybir.dt.bfloat16
AF = mybir.ActivationFunctionType
ALU = mybir.AluOpType

B, C, H, W = 4, 256, 64, 64
N = H * W            # 4096
RC = 32              # reduced channels for q/k
NB = 34 * W          # 2176 band positions (34 rows incl. halo/phantom rows)
PADW = W + 2         # 66
FLAT = 34 * PADW     # 2244 padded band slots
CCH = 2              # channel chunks of 128 (C = 256)
ICH = 4              # conv input-channel chunks of 128 (2C = 512)
MG = N // 128        # 32 m-chunks in PAM contraction
BN_EPS = 1e-5

# PAM/CAM band tiles: (start, len) over the 2176 band positions
PAM_TILES = [(0, 512), (512, 512), (1024, 512), (1536, 512), (2048, 128)]
# conv output tiles: (slot_start, len) over padded flat coords; valid output
# rows are padded rows 1..32 -> flat [66, 2178)
CONV_TILES = [(66, 512), (578, 512), (1090, 512), (1602, 512), (2114, 64)]
# stats chunks over the 2112-long y strip
ST_CHUNKS = [(0, 512), (512, 512), (1024, 512), (1536, 512), (2048, 64)]


def build(n_cores=8, stat_count=4 * N):
    """Build and compile the SPMD kernel graph. Returns compiled Bacc."""
    nc = bacc.Bacc("TRN2", target_bir_lowering=False, debug=False,
                   num_devices=n_cores)

    # ---- DRAM parameters (per core) ----
    x_full = nc.dram_tensor("x_full", [C, N], BF16, kind="ExternalInput")
    x_band = nc.dram_tensor("x_band", [C, NB], BF16, kind="ExternalInput")
    qwT4_d = nc.dram_tensor("qwT4", [C, 128], BF16, kind="ExternalInput")
    kwT4_d = nc.dram_tensor("kwT4", [C, 128], BF16, kind="ExternalInput")
    vwT_d = nc.dram_tensor("vwT", [C, C], BF16, kind="ExternalInput")
    qb4_d = nc.dram_tensor("qb4", [128, 1], F32, kind="ExternalInput")
    kb4_d = nc.dram_tensor("kb4", [128, 1], F32, kind="ExternalInput")
    fw_d = nc.dram_tensor("fw", [ICH, 128, 9 * 2 * 128], BF16, kind="ExternalInput")
    gpinv_d = nc.dram_tensor("gpinv", [128, 1], BF16, kind="ExternalInput")
    gpvb_d = nc.dram_tensor("gpvb", [128, CCH], F32, kind="ExternalInput")
    gc_d = nc.dram_tensor("gc", [128, 1], F32, kind="ExternalInput")
    mf_d = nc.dram_tensor("mf", [128, 1], F32, kind="ExternalInput")   # first-row mask
    ml_d = nc.dram_tensor("ml", [128, 1], F32, kind="ExternalInput")   # last-row mask
    bnsc_d = nc.dram_tensor("bnsc", [128, CCH], F32, kind="ExternalInput")
    bnbi_d = nc.dram_tensor("bnbi", [128, CCH], F32, kind="ExternalInput")
    ones_m1b_d = nc.dram_tensor("ones_m1b", [1, 128], BF16, kind="ExternalInput")
    ident_d = nc.dram_tensor("ident", [128, 128], BF16, kind="ExternalInput")
    y_out = nc.dram_tensor("y_out", [CCH, 128, 32 * W], F32, kind="ExternalOutput")
    st_loc = nc.dram_tensor("st_loc", [128, 4], F32)
    st_glob = nc.dram_tensor("st_glob", [128, 4], F32, addr_space="Shared")

    with tile.TileContext(nc) as tc:
        with tc.tile_pool(name="persist", bufs=1) as pp, \
             tc.tile_pool(name="psum", bufs=1, space="PSUM") as psp:

            # ---- persistent SBUF tiles ----
            k_sb4 = pp.tile([128, N], BF16)        # k, 4 partition-replicas
            q_sb4 = pp.tile([128, NB], BF16)       # q, 4 partition-replicas
            vT = pp.tile([128, MG * C], BF16)      # v^T, m on partitions
            xb = [pp.tile([128, NB], BF16, name=f"xb{i}") for i in range(CCH)]
            qwT4 = pp.tile([128, CCH * 128], BF16)
            kwT4 = pp.tile([128, CCH * 128], BF16)
            vwT = pp.tile([128, CCH * C], BF16)
            qb4 = pp.tile([128, 1], F32)
            kb4 = pp.tile([128, 1], F32)
            gpinv = pp.tile([128, 1], BF16)
            gpvb = pp.tile([128, CCH], F32)
            gc = pp.tile([128, 1], F32)
            mf = pp.tile([128, 1], F32)
            ml = pp.tile([128, 1], F32)
            bnsc = pp.tile([128, CCH], F32)
            bnbi = pp.tile([128, CCH], F32)
            ones_m1b = pp.tile([1, 128], BF16)
            ident = pp.tile([128, 128], BF16)
            fw = [pp.tile([128, 9 * 2 * 128], BF16, name=f"fw{i}") for i in range(ICH)]
            feats = [pp.tile([128, FLAT + 2], BF16, name=f"feats{i}") for i in range(ICH)]
            feats_sh = [pp.tile([128, FLAT + 2], BF16, name=f"feats_sh{i}")
                        for i in range(ICH)]
            y_sb = [pp.tile([128, 32 * PADW], F32, name=f"y_sb{i}") for i in range(CCH)]
            st_sb = pp.tile([128, 4], F32)
            stg = pp.tile([128, 4], F32)
            epsc = pp.tile([128, 1], F32)

            # preload the exp table set while DMAs run
            nc.gpsimd.memset(epsc, BN_EPS)
            dummy = pp.tile([128, 1], F32)
            nc.scalar.activation(dummy, epsc, AF.Exp)

            # ---- chunked input loads: x_band then x_full, 512-col pieces ----
            for i in range(CCH):
                for (s, ln) in PAM_TILES:
                    nc.sync.dma_start(
                        out=xb[i][:, s:s + ln],
                        in_=x_band.ap()[i * 128:(i + 1) * 128, s:s + ln])
            nc.sync.dma_start(out=qwT4[:, 0:128], in_=qwT4_d.ap()[0:128, :])
            nc.sync.dma_start(out=qwT4[:, 128:256], in_=qwT4_d.ap()[128:256, :])
            nc.sync.dma_start(out=kwT4[:, 0:128], in_=kwT4_d.ap()[0:128, :])
            nc.sync.dma_start(out=kwT4[:, 128:256], in_=kwT4_d.ap()[128:256, :])
            nc.sync.dma_start(out=vwT[:, 0:C], in_=vwT_d.ap()[0:128, :])
            nc.sync.dma_start(out=vwT[:, C:2 * C], in_=vwT_d.ap()[128:256, :])
            for t, d in ((qb4, qb4_d), (kb4, kb4_d), (gpinv, gpinv_d),
                         (gpvb, gpvb_d), (gc, gc_d), (mf, mf_d), (ml, ml_d),
                         (bnsc, bnsc_d), (bnbi, bnbi_d),
                         (ones_m1b, ones_m1b_d), (ident, ident_d)):
                nc.sync.dma_start(out=t, in_=d.ap())
            for i in range(ICH):
                nc.sync.dma_start(out=fw[i], in_=fw_d.ap()[i])
            for i in range(ICH):
                nc.gpsimd.memset(feats[i], 0.0)

            # ================= phase 1: xf-derived tensors =================
            with tc.tile_pool(name="early", bufs=1) as ep:
                xf_b = [ep.tile([128, N], BF16, name=f"xf_b{i}") for i in range(CCH)]
                ecp = [psp.tile([128, C], F32, tag="u", bufs=2, name=f"ecp{i}")
                       for i in range(CCH)]

                for t in range(N // 512):
                    for i in range(CCH):
                        nc.sync.dma_start(
                            out=xf_b[i][:, t * 512:(t + 1) * 512],
                            in_=x_full.ap()[i * 128:(i + 1) * 128,
                                            t * 512:(t + 1) * 512])
                    # k projection (4 replicas on partitions), bias on the copy
                    kp = psp.tile([128, 512], F32, tag="wk", bufs=2, name="kp")
                    for ct in range(CCH):
                        nc.tensor.matmul(kp, lhsT=kwT4[:, ct * 128:(ct + 1) * 128],
                                         rhs=xf_b[ct][:, t * 512:(t + 1) * 512],
                                         start=(ct == 0), stop=(ct == CCH - 1))
                    nc.vector.tensor_scalar_add(k_sb4[:, t * 512:(t + 1) * 512],
                                                kp, kb4)
                    # q projection on the band
                    if t < len(PAM_TILES):
                        (sq_, ln) = PAM_TILES[t]
                        qp = psp.tile([128, 512], F32, tag="wk", bufs=2, name="qp")
                        for ct in range(CCH):
                            nc.tensor.matmul(qp[:, 0:ln],
                                             lhsT=qwT4[:, ct * 128:(ct + 1) * 128],
                                             rhs=xb[ct][:, sq_:sq_ + ln],
                                             start=(ct == 0), stop=(ct == CCH - 1))
                        nc.vector.tensor_scalar_add(q_sb4[:, sq_:sq_ + ln],
                                                    qp[:, 0:ln], qb4)
                    for g in range(4 * t, 4 * t + 4):
                        # vT chunk g
                        vp = psp.tile([128, C], F32, tag="wk", bufs=2, name="vp")
                        for ct in range(CCH):
                            nc.tensor.matmul(vp,
                                             lhsT=xf_b[ct][:, g * 128:(g + 1) * 128],
                                             rhs=vwT[:, ct * C:(ct + 1) * C],
                                             start=(ct == 0), stop=(ct == CCH - 1))
                        nc.scalar.activation(vT[:, g * C:(g + 1) * C], vp, AF.Copy)
                        # xfT chunk g + CAM energy accumulation
                        xfT = ep.tile([128, C], BF16, tag="xfT", bufs=3, name="xfT")
                        for ct in range(CCH):
                            tp = psp.tile([128, 128], BF16, tag="et", bufs=2,
                                          name="tp")
                            nc.tensor.transpose(tp, xf_b[ct][:, g * 128:(g + 1) * 128],
                                                ident)
                            nc.vector.tensor_copy(xfT[:, ct * 128:(ct + 1) * 128], tp)
                        for ct in range(CCH):
                            nc.tensor.matmul(ecp[ct],
                                             lhsT=xfT[:, ct * 128:(ct + 1) * 128],
                                             rhs=xfT, start=(g == 0),
                                             stop=(g == MG - 1))

                # ---- phase 2: CAM softmax + apply (gamma_c folded in) ----
                attn_cT = ep.tile([128, CCH * C], BF16)
                for ct in range(CCH):
                    emin = ep.tile([128, 1], F32, tag="cam_t", bufs=8, name="emin")
                    esum = ep.tile([128, 1], F32, tag="cam_t", bufs=8, name="esum")
                    erec = ep.tile([128, 1], F32, tag="cam_t", bufs=8, name="erec")
                    ea = ep.tile([128, C], F32, tag="ea", bufs=2, name="ea")
                    attn_c = ep.tile([128, C], BF16, tag="attn_c", bufs=2,
                                     name="attn_c")
                    nc.vector.tensor_reduce(emin, ecp[ct], axis=mybir.AxisListType.X,
                                            op=ALU.min)
                    nc.scalar.activation(ea, ecp[ct], AF.Exp, bias=emin, scale=-1.0,
                                         accum_out=esum)
                    nc.vector.reciprocal(erec, esum)
                    # attn_c = (ea * erec) * gamma_c
                    nc.vector.tensor_scalar(attn_c, ea, erec, gc,
                                            op0=ALU.mult, op1=ALU.mult)
                    for dc in range(CCH):
                        tp2 = psp.tile([128, 128], BF16, tag="et", bufs=2, name="tp2")
                        nc.tensor.transpose(tp2, attn_c[:, dc * 128:(dc + 1) * 128],
                                            ident)
                        nc.vector.tensor_copy(
                            attn_cT[:, (dc * CCH + ct) * 128:
                                    (dc * CCH + ct + 1) * 128], tp2)

                # cam_out = attn_c @ x_band; channel = cam + x_band -> feats[2..3]
                for ct in range(CCH):
                    for (s, ln) in PAM_TILES:
                        cp = psp.tile([128, 512], F32, tag="wk", bufs=2, name="cp")
                        for dc in range(CCH):
                            nc.tensor.matmul(
                                cp[:, 0:ln],
                                lhsT=attn_cT[:, (dc * CCH + ct) * 128:
                                             (dc * CCH + ct + 1) * 128],
                                rhs=xb[dc][:, s:s + ln],
                                start=(dc == 0), stop=(dc == CCH - 1))
                        r0, nr = s // W, ln // W
                        fdst = feats[CCH + ct][:, 2:2 + 34 * PADW] \
                            .rearrange("p (r w) -> p r w", w=PADW)[:, r0:r0 + nr, 0:W]
                        fsrc_cam = cp[:, 0:ln].rearrange("p (r w) -> p r w", w=W)
                        fx = xb[ct][:, s:s + ln].rearrange("p (r w) -> p r w", w=W)
                        nc.vector.tensor_add(fdst, fsrc_cam, fx)

            # ============ phases 3-5 use the late pool (reuses early space) ====
            with tc.tile_pool(name="late", bufs=1) as lp:
                # ---- phase 3: PAM pipeline ----
                def pam_tail(ti, s, ln, u, s_acc):
                    # softmax denominator: rb = broadcast(gp / s)
                    s512 = lp.tile([128, 512], BF16, tag="s512", bufs=2, name="s512")
                    nc.vector.tensor_add(s512[:, 0:ln], s_acc[:, 0:ln],
                                         s_acc[:, ln:2 * ln])
                    ssum = psp.tile([1, 512], F32, tag="wk", bufs=2, name="ssum")
                    nc.tensor.matmul(ssum[:, 0:ln], lhsT=gpinv, rhs=s512[:, 0:ln],
                                     start=True, stop=True)
                    ssb = lp.tile([1, 512], BF16, tag="ssb", bufs=2, name="ssb")
                    nc.vector.tensor_copy(ssb[:, 0:ln], ssum[:, 0:ln])
                    rb = psp.tile([128, 512], F32, tag="wk", bufs=2, name="rb")
                    nc.tensor.matmul(rb[:, 0:ln], lhsT=ones_m1b, rhs=ssb[:, 0:ln],
                                     start=True, stop=True)
                    rb_sb = lp.tile([128, 512], F32, tag="rb_sb", bufs=2,
                                    name="rb_sb")
                    nc.vector.reciprocal_approx_fast(rb_sb[:, 0:ln], rb[:, 0:ln])
                    # position = (u * gp/s + gp*vb) + x_band -> feats[0..1]
                    r0, nr = s // W, ln // W
                    for ct in range(CCH):
                        tmp = lp.tile([128, 512], F32, tag="tmp", bufs=2, name="tmp")
                        nc.vector.tensor_mul(tmp[:, 0:ln], u[ct][:, 0:ln],
                                             rb_sb[:, 0:ln])
                        fdst = feats[ct][:, 2:2 + 34 * PADW] \
                            .rearrange("p (r w) -> p r w", w=PADW)[:, r0:r0 + nr, 0:W]
                        fsrc = tmp[:, 0:ln].rearrange("p (r w) -> p r w", w=W)
                        fx = xb[ct][:, s:s + ln].rearrange("p (r w) -> p r w", w=W)
                        nc.vector.scalar_tensor_tensor(fdst, fsrc,
                                                       gpvb[:, ct:ct + 1], fx,
                                                       op0=ALU.add, op1=ALU.add)
                    # phantom halo rows: row 0 after first tile, row 33 after last
                    if ti == 0:
                        for i in range(ICH):
                            fv = feats[i][:, 2:2 + 34 * PADW] \
                                .rearrange("p (r w) -> p r w", w=PADW)
                            nc.vector.tensor_scalar_mul(fv[:, 0:1, 0:W],
                                                        fv[:, 0:1, 0:W], mf)
                    if ti == len(PAM_TILES) - 1:
                        for i in range(ICH):
                            fv = feats[i][:, 2:2 + 34 * PADW] \
                                .rearrange("p (r w) -> p r w", w=PADW)
                            nc.vector.tensor_scalar_mul(fv[:, 33:34, 0:W],
                                                        fv[:, 33:34, 0:W], ml)

                pend = None
                for ti, (s, ln) in enumerate(PAM_TILES):
                    u = [psp.tile([128, 512], F32, tag="u", bufs=2, name=f"u{i}")
                         for i in range(CCH)]
                    s_acc = lp.tile([128, 1024], BF16, tag="s_acc", bufs=2,
                                    name="s_acc")
                    for qt in range(4):      # quarter-tiles of 8 m-chunks
                        esb = lp.tile([128, 8 * 512], BF16, tag="esb", bufs=3,
                                      name="esb")
                        for p in range(4):   # pairs of m-chunks
                            g0 = qt * 8 + p * 2
                            rg = (0, 32) if p % 2 == 0 else (64, 96)
                            et = psp.tile([128, 1024], F32, tag="et", bufs=2,
                                          name="et")
                            for b in range(2):
                                gb = g0 + b
                                nc.tensor.matmul(
                                    et[:, b * 512:b * 512 + ln],
                                    lhsT=k_sb4[rg[b]:rg[b] + 32,
                                               gb * 128:(gb + 1) * 128],
                                    rhs=q_sb4[rg[b]:rg[b] + 32, s:s + ln],
                                    start=True, stop=True,
                                    tile_position=(rg[b], 0))
                            eo = p * 2 * ln
                            if ln == 512:
                                nc.scalar.activation(esb[:, eo:eo + 1024], et,
                                                     AF.Exp)
                            else:
                                ev = et.rearrange("p (b x) -> p b x", b=2)[:, :, 0:ln]
                                nc.scalar.activation(
                                    esb[:, eo:eo + 2 * ln]
                                    .rearrange("p (b x) -> p b x", b=2), ev, AF.Exp)
                            # denominator partial sums on DVE
                            if qt == 0 and p == 0:
                                nc.vector.tensor_copy(s_acc[:, 0:2 * ln],
                                                      esb[:, eo:eo + 2 * ln])
                            else:
                                nc.vector.tensor_add(s_acc[:, 0:2 * ln],
                                                     s_acc[:, 0:2 * ln],
                                                     esb[:, eo:eo + 2 * ln])
                        # dense apply runs for this quarter
                        for ct in range(CCH):
                            for gl in range(8):
                                g = qt * 8 + gl
                                nc.tensor.matmul(
                                    u[ct][:, 0:ln],
                                    lhsT=vT[:, g * C + ct * 128:
                                            g * C + (ct + 1) * 128],
                                    rhs=esb[:, gl * ln:(gl + 1) * ln],
                                    start=(g == 0), stop=(g == MG - 1))
                        # previous tile's combine, emitted one quarter late so
                        # this tile's energy matmuls precede it in PE priority
                        if qt == 0 and pend is not None:
                            pam_tail(*pend)
                    pend = (ti, s, ln, u, s_acc)
                pam_tail(*pend)

                # shifted copy so odd-offset conv taps read 4-byte-aligned
                for i in range(ICH):
                    nc.vector.tensor_copy(feats_sh[i][:, 0:FLAT + 1],
                                          feats[i][:, 1:FLAT + 2])

                # ---- phase 4+5: conv3x3 with fused BN stats, per-ot
                # AllReduce so chunk 0's BN/relu/store hides under chunk 1 ----
                inv_n = 1.0 / float(stat_count)
                for ot in range(CCH):
                    sparts = [lp.tile([128, len(CONV_TILES)], F32, tag="sparts",
                                      bufs=4, name=f"sparts{i}") for i in range(2)]
                    for ci, (s, ln) in enumerate(CONV_TILES):
                        yp = psp.tile([128, 512], F32, tag="u", bufs=2, name="yp")
                        first = True
                        for tap in range(9):
                            off = (tap // 3 - 1) * PADW + (tap % 3 - 1)
                            o = 1 + s + off
                            for ic in range(ICH):
                                src = feats[ic][:, o:o + ln] if o % 2 == 0 \
                                    else feats_sh[ic][:, o - 1:o - 1 + ln]
                                nc.tensor.matmul(
                                    yp[:, 0:ln],
                                    lhsT=fw[ic][:, (tap * 2 + ot) * 128:
                                                (tap * 2 + ot + 1) * 128],
                                    rhs=src,
                                    start=first, stop=(tap == 8 and ic == ICH - 1))
                                first = False
                        a = s - 66
                        nc.vector.tensor_copy(y_sb[ot][:, a:a + ln], yp[:, 0:ln])
                        # zero the junk pad cols inside this strip, then stats
                        yv = y_sb[ot].rearrange("p (r w) -> p r w", w=PADW)
                        for c in (0, 65):
                            r0 = max(0, (a - c + PADW - 1) // PADW)
                            r1 = min(32, (a + ln - 1 - c) // PADW + 1)
                            if r1 > r0:
                                nc.vector.memset(yv[:, r0:r1, c:c + 1], 0.0)
                        nc.vector.tensor_reduce(sparts[0][:, ci:ci + 1],
                                                y_sb[ot][:, a:a + ln],
                                                axis=mybir.AxisListType.X, op=ALU.add)
                        sq = lp.tile([128, 512], F32, tag="sq", bufs=2, name="sq")
                        nc.scalar.activation(sq[:, 0:ln], y_sb[ot][:, a:a + ln],
                                             AF.Square,
                                             accum_out=sparts[1][:, ci:ci + 1])
                    for i in range(2):
                        nc.vector.tensor_reduce(st_sb[:, 2 * ot + i:2 * ot + i + 1],
                                                sparts[i],
                                                axis=mybir.AxisListType.X, op=ALU.add)

                # ---- phase 5: one AllReduce, BN math, relu, store ----
                nc.sync.dma_start(out=st_loc.ap(), in_=st_sb)
                if n_cores > 1:
                    nc.gpsimd.collective_compute(
                        "AllReduce", ALU.add,
                        replica_groups=[list(range(n_cores))],
                        ins=[st_loc.ap()], outs=[st_glob.ap()])
                else:
                    nc.gpsimd.dma_start(out=st_glob.ap(), in_=st_loc.ap())
                nc.sync.dma_start(out=stg, in_=st_glob.ap())

                # BN math for both channel chunks at once ([128, 2] columns)
                mean = lp.tile([128, 2], F32, tag="bn_t", bufs=8, name="mean")
                msq = lp.tile([128, 2], F32, tag="bn_t", bufs=8, name="msq")
                var = lp.tile([128, 2], F32, tag="bn_t", bufs=8, name="var")
                m2 = lp.tile([128, 2], F32, tag="bn_t", bufs=8, name="m2")
                std = lp.tile([128, 2], F32, tag="bn_t", bufs=8, name="std")
                rstd = lp.tile([128, 2], F32, tag="bn_t", bufs=8, name="rstd")
                sc2 = lp.tile([128, 2], F32, tag="bn_t", bufs=8, name="sc2")
                bi2 = lp.tile([128, 2], F32, tag="bn_t", bufs=8, name="bi2")
                t0 = lp.tile([128, 2], F32, tag="bn_t", bufs=8, name="t0")
                stv = stg.rearrange("p (o i) -> p o i", i=2)
                nc.vector.tensor_scalar_mul(mean, stv[:, :, 0], inv_n)
                nc.vector.tensor_scalar_mul(msq, stv[:, :, 1], inv_n)
                nc.vector.tensor_mul(m2, mean, mean)
                nc.vector.tensor_sub(var, msq, m2)
                nc.scalar.activation(std, var, AF.Sqrt, bias=epsc)
                nc.vector.reciprocal(rstd, std)
                nc.vector.tensor_mul(sc2, bnsc, rstd)
                nc.vector.tensor_mul(t0, mean, sc2)
                nc.vector.tensor_sub(bi2, bnbi, t0)
                # y = relu(y*scale' + bias') on valid cols, then store
                for ot in range(CCH):
                    stage = lp.tile([128, 32 * W], F32, tag="stage", bufs=4,
                                    name="stage")
                    yvv = y_sb[ot].rearrange("p (r w) -> p r w", w=PADW)
                    for hf in range(2):
                        ysrc = yvv[:, hf * 16:hf * 16 + 16, 1:65]
                        nc.scalar.activation(
                            stage[:, hf * 1024:(hf + 1) * 1024]
                            .rearrange("p (r w) -> p r w", w=W), ysrc, AF.Relu,
                            bias=bi2[:, ot:ot + 1], scale=sc2[:, ot:ot + 1])
                        for dq in range(2):
                            o = hf * 1024 + dq * 512
                            nc.sync.dma_start(
                                out=y_out.ap()[ot][:, o:o + 512],
                                in_=stage[:, o:o + 512])

    nc.compile()
    return nc


_CACHE = {}


def _get_nc(n_cores=8, stat_count=4 * N):
    key = (n_cores, stat_count)
    if key not in _CACHE:
        _CACHE[key] = build(n_cores, stat_count)
    return _CACHE[key]


def make_in_maps(x, q_w, q_b, k_w, k_b, v_w, v_b, gamma_p, gamma_c,
                 fuse_w, bn_scale, bn_bias, cores=8):
    f4 = np.float32
    shared = {}
    # 4 partition-replicas of the q/k projection weights and biases
    qwT = np.asarray(q_w, f4).T                  # [C, RC]
    kwT = np.asarray(k_w, f4).T
    shared["qwT4"] = np.ascontiguousarray(np.tile(qwT, (1, 4))) \
        .astype(ml_dtypes.bfloat16)              # [C, 128]
    shared["kwT4"] = np.ascontiguousarray(np.tile(kwT, (1, 4))) \
        .astype(ml_dtypes.bfloat16)
    shared["qb4"] = np.tile(np.asarray(q_b, f4), 4).reshape(128, 1)
    shared["kb4"] = np.tile(np.asarray(k_b, f4), 4).reshape(128, 1)
    shared["vwT"] = np.ascontiguousarray(np.asarray(v_w, f4).T).astype(ml_dtypes.bfloat16)
    # fuse_w [256, 512, 3, 3] -> [ic, i, tap*2*128 + ot*128 + o] in bf16
    fwr = np.asarray(fuse_w, f4).reshape(CCH, 128, ICH, 128, 3, 3)
    fwt = np.ascontiguousarray(fwr.transpose(2, 3, 4, 5, 0, 1))  # ic,i,kh,kw,ot,o
    shared["fw"] = fwt.reshape(ICH, 128, 9 * 2 * 128).astype(ml_dtypes.bfloat16)
    gp = np.asarray(gamma_p, f4).ravel()[0]
    gcv = np.asarray(gamma_c, f4).ravel()[0]
    with np.errstate(divide="ignore"):
        gpi = np.float32(1.0) / gp
    shared["gpinv"] = np.full((128, 1), gpi, f4).astype(ml_dtypes.bfloat16)
    shared["gpvb"] = np.ascontiguousarray(
        (gp * np.asarray(v_b, f4)).reshape(CCH, 128).T)
    shared["gc"] = np.full((128, 1), gcv, f4)
    shared["bnsc"] = np.ascontiguousarray(np.asarray(bn_scale, f4).reshape(CCH, 128).T)
    shared["bnbi"] = np.ascontiguousarray(np.asarray(bn_bias, f4).reshape(CCH, 128).T)
    shared["ones_m1b"] = np.ones((1, 128), ml_dtypes.bfloat16)
    shared["ident"] = np.eye(128, dtype=ml_dtypes.bfloat16)

    xs = np.asarray(x, f4).reshape(B, C, N).astype(ml_dtypes.bfloat16)
    zrow = np.zeros((C, W), ml_dtypes.bfloat16)
    in_maps = []
    for core in range(cores):
        b, j = core // 2, core % 2
        xf = np.ascontiguousarray(xs[b])
        if j == 0:
            band = np.concatenate([zrow, xf[:, 0:33 * W]], axis=1)
            mfv, mlv = 0.0, 1.0
        else:
            band = np.concatenate([xf[:, 31 * W:], zrow], axis=1)
            mfv, mlv = 1.0, 0.0
        m = dict(shared)
        m["x_full"] = xf
        m["x_band"] = np.ascontiguousarray(band)
        m["mf"] = np.full((128, 1), mfv, f4)
        m["ml"] = np.full((128, 1), mlv, f4)
        in_maps.append(m)
    return in_maps


def kernel(**inputs):
    nc = _get_nc(8)
    in_maps = make_in_maps(**inputs)
    r = run_bass_kernel_spmd(nc, in_maps, core_ids=list(range(8)))
    out = np.empty((B, C, H, W), np.float32)
    for core in range(8):
        b, j = core // 2, core % 2
        y2 = r.results[core]["y_out"]  # [2, 128, 2048]
        out[b, 0:128, 32 * j:32 * j + 32, :] = y2[0].reshape(128, 32, W)
        out[b, 128:256, 32 * j:32 * j + 32, :] = y2[1].reshape(128, 32, W)
    return out


# revision 26
# speedup vs baseline: 1.0913x; 1.0071x over previous
"""DANet attention (PAM + CAM + fuse conv3x3 + BN + ReLU) on 8 TRN2 NeuronCores.

Sharding: core = 2*b + j handles sample b (of 4) and spatial band j (of 2).
Each band covers 34 rows of the 64-row image (32 output rows + 1 halo row on
each side; out-of-image halo rows are zero). PAM/CAM are computed for the
band's 2176 positions against the full 4096-position sample; the 3x3 fuse
conv runs on a zero-padded 34x66 layout; BN batch stats are combined across
all 8 cores with a single tiny AllReduce.

Perf structure (v2):
- chunked input DMAs so compute starts early
- k/q projections emit 4 partition-replicas (host-replicated weights) so the
  K=32 PAM energy matmuls can be row-tiled two-at-a-time into one 2-bank
  PSUM tile; exp processes [128, 2*ln] per pair on the scalar engine
- exp results staged in SBUF per quarter-tile (8 m-chunks), PAM apply
  matmuls run as dense 8-MM accumulation bursts that pipeline against the
  next quarter's energy/exp work
- biases/gammas folded: q/k bias via DVE tensor_scalar_add on the PSUM
  evacuation; gamma_p via 1/gp in the softmax denominator reduction and a
  host-side gp*v_b column in the final combine; gamma_c into attn_c
- BN stats for both channel chunks combined into one AllReduce

Self-contained: hardcodes shapes B=4, C=256, H=W=64, RC=32.
"""

import numpy as np
import ml_dtypes

import concourse.bass as bass
import concourse.tile as tile
from concourse import bacc, mybir
from concourse.bass_utils import run_bass_kernel_spmd
F32 = mybir.dt.float32
F32R = mybir.dt.float32r
BF16 = mybir.dt.bfloat16
AF = mybir.ActivationFunctionType
ALU = mybir.AluOpType

B, C, H, W = 4, 256, 64, 64
N = H * W            # 4096
RC = 32              # reduced channels for q/k
NB = 34 * W          # 2176 band positions (34 rows incl. halo/phantom rows)
PADW = W + 2         # 66
FLAT = 34 * PADW     # 2244 padded band slots
CCH = 2              # channel chunks of 128 (C = 256)
ICH = 4              # conv input-channel chunks of 128 (2C = 512)
MG = N // 128        # 32 m-chunks in PAM contraction
BN_EPS = 1e-5

# PAM/CAM band tiles: (start, len) over the 2176 band positions
PAM_TILES = [(0, 512), (512, 512), (1024, 512), (1536, 512), (2048, 128)]
# conv output tiles: (slot_start, len) over padded flat coords; valid output
# rows are padded rows 1..32 -> flat [66, 2178)
CONV_TILES = [(66, 512), (578, 512), (1090, 512), (1602, 512), (2114, 64)]
# stats chunks over the 2112-long y strip
ST_CHUNKS = [(0, 512), (512, 512), (1024, 512), (1536, 512), (2048, 64)]


def build(n_cores=8, stat_count=4 * N):
    """Build and compile the SPMD kernel graph. Returns compiled Bacc."""
    nc = bacc.Bacc("TRN2", target_bir_lowering=False, debug=False,
                   num_devices=n_cores)

    # ---- DRAM parameters (per core) ----
    x_full = nc.dram_tensor("x_full", [C, N], BF16, kind="ExternalInput")
    x_band = nc.dram_tensor("x_band", [C, NB], BF16, kind="ExternalInput")
    qwT4_d = nc.dram_tensor("qwT4", [C, 128], BF16, kind="ExternalInput")
    kwT4_d = nc.dram_tensor("kwT4", [C, 128], BF16, kind="ExternalInput")
    vwT_d = nc.dram_tensor("vwT", [C, C], BF16, kind="ExternalInput")
    qb4_d = nc.dram_tensor("qb4", [128, 1], F32, kind="ExternalInput")
    kb4_d = nc.dram_tensor("kb4", [128, 1], F32, kind="ExternalInput")
    fw_d = nc.dram_tensor("fw", [ICH, 128, 9 * 2 * 128], BF16, kind="ExternalInput")
    gpinv_d = nc.dram_tensor("gpinv", [128, 1], BF16, kind="ExternalInput")
    gpvb_d = nc.dram_tensor("gpvb", [128, CCH], F32, kind="ExternalInput")
    gc_d = nc.dram_tensor("gc", [128, 1], F32, kind="ExternalInput")
    mf_d = nc.dram_tensor("mf", [128, 1], F32, kind="ExternalInput")   # first-row mask
    ml_d = nc.dram_tensor("ml", [128, 1], F32, kind="ExternalInput")   # last-row mask
    bnsc_d = nc.dram_tensor("bnsc", [128, CCH], F32, kind="ExternalInput")
    bnbi_d = nc.dram_tensor("bnbi", [128, CCH], F32, kind="ExternalInput")
    ones_m1b_d = nc.dram_tensor("ones_m1b", [1, 128], BF16, kind="ExternalInput")
    ident_d = nc.dram_tensor("ident", [128, 128], BF16, kind="ExternalInput")
    y_out = nc.dram_tensor("y_out", [CCH, 128, 32 * W], F32, kind="ExternalOutput")
    st_loc = nc.dram_tensor("st_loc", [128, 4], F32)
    st_glob = nc.dram_tensor("st_glob", [128, 4], F32, addr_space="Shared")

    with tile.TileContext(nc) as tc:
        with tc.tile_pool(name="persist", bufs=1) as pp, \
             tc.tile_pool(name="psum", bufs=1, space="PSUM") as psp:

            # ---- persistent SBUF tiles ----
            k_sb4 = pp.tile([128, N], BF16)        # k, 4 partition-replicas
            q_sb4 = pp.tile([128, NB], BF16)       # q, 4 partition-replicas
            vT = pp.tile([128, MG * C], BF16)      # v^T, m on partitions
            xb = [pp.tile([128, NB], BF16, name=f"xb{i}") for i in range(CCH)]
            qwT4 = pp.tile([128, CCH * 128], BF16)
            kwT4 = pp.tile([128, CCH * 128], BF16)
            vwT = pp.tile([128, CCH * C], BF16)
            qb4 = pp.tile([128, 1], F32)
            kb4 = pp.tile([128, 1], F32)
            gpinv = pp.tile([128, 1], BF16)
            gpvb = pp.tile([128, CCH], F32)
            gc = pp.tile([128, 1], F32)
            mf = pp.tile([128, 1], F32)
            ml = pp.tile([128, 1], F32)
            bnsc = pp.tile([128, CCH], F32)
            bnbi = pp.tile([128, CCH], F32)
            ones_m1b = pp.tile([1, 128], BF16)
            ident = pp.tile([128, 128], BF16)
            fw = [pp.tile([128, 9 * 2 * 128], BF16, name=f"fw{i}") for i in range(ICH)]
            feats = [pp.tile([128, FLAT + 2], BF16, name=f"feats{i}") for i in range(ICH)]
            feats_sh = [pp.tile([128, FLAT + 2], BF16, name=f"feats_sh{i}")
                        for i in range(ICH)]
            y_sb = [pp.tile([128, 32 * PADW], F32, name=f"y_sb{i}") for i in range(CCH)]
            st_sb = pp.tile([128, 4], F32)
            stg = pp.tile([128, 4], F32)
            epsc = pp.tile([128, 1], F32)

            # preload the exp table set while DMAs run
            nc.gpsimd.memset(epsc, BN_EPS)
            dummy = pp.tile([128, 1], F32)
            nc.scalar.activation(dummy, epsc, AF.Exp)

            # ---- input loads: small weights first so compute starts early ----
            nc.sync.dma_start(out=kwT4[:, 0:128], in_=kwT4_d.ap()[0:128, :])
            nc.sync.dma_start(out=kwT4[:, 128:256], in_=kwT4_d.ap()[128:256, :])
            nc.sync.dma_start(out=qwT4[:, 0:128], in_=qwT4_d.ap()[0:128, :])
            nc.sync.dma_start(out=qwT4[:, 128:256], in_=qwT4_d.ap()[128:256, :])
            nc.sync.dma_start(out=vwT[:, 0:C], in_=vwT_d.ap()[0:128, :])
            nc.sync.dma_start(out=vwT[:, C:2 * C], in_=vwT_d.ap()[128:256, :])
            for t, d in ((qb4, qb4_d), (kb4, kb4_d), (gpinv, gpinv_d),
                         (gpvb, gpvb_d), (gc, gc_d), (mf, mf_d), (ml, ml_d),
                         (bnsc, bnsc_d), (bnbi, bnbi_d),
                         (ones_m1b, ones_m1b_d), (ident, ident_d)):
                nc.sync.dma_start(out=t, in_=d.ap())
            # ================= phase 1: xf-derived tensors =================
            with tc.tile_pool(name="early", bufs=1) as ep:
                xf_b = [ep.tile([128, N], BF16, name=f"xf_b{i}") for i in range(CCH)]
                ecp = [psp.tile([128, C], F32, tag="u", bufs=2, name=f"ecp{i}")
                       for i in range(CCH)]

                for t in range(N // 512):
                    for i in range(CCH):
                        nc.sync.dma_start(
                            out=xf_b[i][:, t * 512:(t + 1) * 512],
                            in_=x_full.ap()[i * 128:(i + 1) * 128,
                                            t * 512:(t + 1) * 512])
                # x band chunks (q-proj and CAM/PAM combines read these)
                for i in range(CCH):
                    for (s, ln) in PAM_TILES:
                        nc.sync.dma_start(
                            out=xb[i][:, s:s + ln],
                            in_=x_band.ap()[i * 128:(i + 1) * 128, s:s + ln])
                for i in range(ICH):
                    nc.sync.dma_start(out=fw[i], in_=fw_d.ap()[i])
                for i in range(ICH):
                    nc.gpsimd.memset(feats[i], 0.0)

                for t in range(N // 512):
                    # k projection (4 replicas on partitions), bias on the copy
                    kp = psp.tile([128, 512], F32, tag="wk", bufs=2, name="kp")
                    for ct in range(CCH):
                        nc.tensor.matmul(kp, lhsT=kwT4[:, ct * 128:(ct + 1) * 128],
                                         rhs=xf_b[ct][:, t * 512:(t + 1) * 512],
                                         start=(ct == 0), stop=(ct == CCH - 1))
                    nc.vector.tensor_scalar_add(k_sb4[:, t * 512:(t + 1) * 512],
                                                kp, kb4)
                    # q projection on the band
                    if t < len(PAM_TILES):
                        (sq_, ln) = PAM_TILES[t]
                        qp = psp.tile([128, 512], F32, tag="wk", bufs=2, name="qp")
                        for ct in range(CCH):
                            nc.tensor.matmul(qp[:, 0:ln],
                                             lhsT=qwT4[:, ct * 128:(ct + 1) * 128],
                                             rhs=xb[ct][:, sq_:sq_ + ln],
                                             start=(ct == 0), stop=(ct == CCH - 1))
                        nc.vector.tensor_scalar_add(q_sb4[:, sq_:sq_ + ln],
                                                    qp[:, 0:ln], qb4)
                    for g in range(4 * t, 4 * t + 4):
                        # vT chunk g
                        vp = psp.tile([128, C], F32, tag="wk", bufs=2, name="vp")
                        for ct in range(CCH):
                            nc.tensor.matmul(vp,
                                             lhsT=xf_b[ct][:, g * 128:(g + 1) * 128],
                                             rhs=vwT[:, ct * C:(ct + 1) * C],
                                             start=(ct == 0), stop=(ct == CCH - 1))
                        nc.scalar.activation(vT[:, g * C:(g + 1) * C], vp, AF.Copy)
                        # xfT chunk g + CAM energy accumulation
                        xfT = ep.tile([128, C], BF16, tag="xfT", bufs=3, name="xfT")
                        for ct in range(CCH):
                            tp = psp.tile([128, 128], BF16, tag="et", bufs=2,
                                          name="tp")
                            nc.tensor.transpose(tp, xf_b[ct][:, g * 128:(g + 1) * 128],
                                                ident)
                            nc.vector.tensor_copy(xfT[:, ct * 128:(ct + 1) * 128], tp)
                        for ct in range(CCH):
                            nc.tensor.matmul(ecp[ct],
                                             lhsT=xfT[:, ct * 128:(ct + 1) * 128],
                                             rhs=xfT, start=(g == 0),
                                             stop=(g == MG - 1))

                # ---- phase 2: CAM softmax + apply (gamma_c folded in) ----
                attn_cT = ep.tile([128, CCH * C], BF16)
                for ct in range(CCH):
                    emin = ep.tile([128, 1], F32, tag="cam_t", bufs=8, name="emin")
                    esum = ep.tile([128, 1], F32, tag="cam_t", bufs=8, name="esum")
                    erec = ep.tile([128, 1], F32, tag="cam_t", bufs=8, name="erec")
                    ea = ep.tile([128, C], F32, tag="ea", bufs=2, name="ea")
                    attn_c = ep.tile([128, C], BF16, tag="attn_c", bufs=2,
                                     name="attn_c")
                    nc.vector.tensor_reduce(emin, ecp[ct], axis=mybir.AxisListType.X,
                                            op=ALU.min)
                    nc.scalar.activation(ea, ecp[ct], AF.Exp, bias=emin, scale=-1.0,
                                         accum_out=esum)
                    nc.vector.reciprocal(erec, esum)
                    # attn_c = (ea * erec) * gamma_c
                    nc.vector.tensor_scalar(attn_c, ea, erec, gc,
                                            op0=ALU.mult, op1=ALU.mult)
                    for dc in range(CCH):
                        tp2 = psp.tile([128, 128], BF16, tag="et", bufs=2, name="tp2")
                        nc.tensor.transpose(tp2, attn_c[:, dc * 128:(dc + 1) * 128],
                                            ident)
                        nc.vector.tensor_copy(
                            attn_cT[:, (dc * CCH + ct) * 128:
                                    (dc * CCH + ct + 1) * 128], tp2)

                # cam_out = attn_c @ x_band; channel = cam + x_band -> feats[2..3]
                for ct in range(CCH):
                    for (s, ln) in PAM_TILES:
                        cp = psp.tile([128, 512], F32, tag="wk", bufs=2, name="cp")
                        for dc in range(CCH):
                            nc.tensor.matmul(
                                cp[:, 0:ln],
                                lhsT=attn_cT[:, (dc * CCH + ct) * 128:
                                             (dc * CCH + ct + 1) * 128],
                                rhs=xb[dc][:, s:s + ln],
                                start=(dc == 0), stop=(dc == CCH - 1))
                        r0, nr = s // W, ln // W
                        fdst = feats[CCH + ct][:, 2:2 + 34 * PADW] \
                            .rearrange("p (r w) -> p r w", w=PADW)[:, r0:r0 + nr, 0:W]
                        fsrc_cam = cp[:, 0:ln].rearrange("p (r w) -> p r w", w=W)
                        fx = xb[ct][:, s:s + ln].rearrange("p (r w) -> p r w", w=W)
                        nc.vector.tensor_add(fdst, fsrc_cam, fx)

            # ============ phases 3-5 use the late pool (reuses early space) ====
            with tc.tile_pool(name="late", bufs=1) as lp:
                # ---- phase 3: PAM pipeline ----
                def pam_tail(ti, s, ln, u, s_acc):
                    # softmax denominator: rb = broadcast(gp / s)
                    s512 = lp.tile([128, 512], BF16, tag="s512", bufs=2, name="s512")
                    nc.vector.tensor_add(s512[:, 0:ln], s_acc[:, 0:ln],
                                         s_acc[:, ln:2 * ln])
                    ssum = psp.tile([1, 512], F32, tag="wk", bufs=2, name="ssum")
                    nc.tensor.matmul(ssum[:, 0:ln], lhsT=gpinv, rhs=s512[:, 0:ln],
                                     start=True, stop=True)
                    ssb = lp.tile([1, 512], BF16, tag="ssb", bufs=2, name="ssb")
                    nc.vector.tensor_copy(ssb[:, 0:ln], ssum[:, 0:ln])
                    rb = psp.tile([128, 512], F32, tag="wk", bufs=2, name="rb")
                    nc.tensor.matmul(rb[:, 0:ln], lhsT=ones_m1b, rhs=ssb[:, 0:ln],
                                     start=True, stop=True)
                    rb_sb = lp.tile([128, 512], F32, tag="rb_sb", bufs=2,
                                    name="rb_sb")
                    nc.vector.reciprocal_approx_fast(rb_sb[:, 0:ln], rb[:, 0:ln])
                    # position = (u * gp/s + gp*vb) + x_band -> feats[0..1]
                    r0, nr = s // W, ln // W
                    for ct in range(CCH):
                        tmp = lp.tile([128, 512], F32, tag="tmp", bufs=2, name="tmp")
                        nc.vector.tensor_mul(tmp[:, 0:ln], u[ct][:, 0:ln],
                                             rb_sb[:, 0:ln])
                        fdst = feats[ct][:, 2:2 + 34 * PADW] \
                            .rearrange("p (r w) -> p r w", w=PADW)[:, r0:r0 + nr, 0:W]
                        fsrc = tmp[:, 0:ln].rearrange("p (r w) -> p r w", w=W)
                        fx = xb[ct][:, s:s + ln].rearrange("p (r w) -> p r w", w=W)
                        nc.vector.scalar_tensor_tensor(fdst, fsrc,
                                                       gpvb[:, ct:ct + 1], fx,
                                                       op0=ALU.add, op1=ALU.add)
                    # phantom halo rows: row 0 after first tile, row 33 after last
                    if ti == 0:
                        for i in range(ICH):
                            fv = feats[i][:, 2:2 + 34 * PADW] \
                                .rearrange("p (r w) -> p r w", w=PADW)
                            nc.vector.tensor_scalar_mul(fv[:, 0:1, 0:W],
                                                        fv[:, 0:1, 0:W], mf)
                    if ti == len(PAM_TILES) - 1:
                        for i in range(ICH):
                            fv = feats[i][:, 2:2 + 34 * PADW] \
                                .rearrange("p (r w) -> p r w", w=PADW)
                            nc.vector.tensor_scalar_mul(fv[:, 33:34, 0:W],
                                                        fv[:, 33:34, 0:W], ml)

                pend = None
                for ti, (s, ln) in enumerate(PAM_TILES):
                    u = [psp.tile([128, 512], F32, tag="u", bufs=2, name=f"u{i}")
                         for i in range(CCH)]
                    s_acc = lp.tile([128, 1024], BF16, tag="s_acc", bufs=2,
                                    name="s_acc")
                    for qt in range(4):      # quarter-tiles of 8 m-chunks
                        esb = lp.tile([128, 8 * 512], BF16, tag="esb", bufs=3,
                                      name="esb")
                        for p in range(4):   # pairs of m-chunks
                            g0 = qt * 8 + p * 2
                            rg = (0, 32) if p % 2 == 0 else (64, 96)
                            et = psp.tile([128, 1024], F32, tag="et", bufs=2,
                                          name="et")
                            for b in range(2):
                                gb = g0 + b
                                nc.tensor.matmul(
                                    et[:, b * 512:b * 512 + ln],
                                    lhsT=k_sb4[rg[b]:rg[b] + 32,
                                               gb * 128:(gb + 1) * 128],
                                    rhs=q_sb4[rg[b]:rg[b] + 32, s:s + ln],
                                    start=True, stop=True,
                                    tile_position=(rg[b], 0))
                            eo = p * 2 * ln
                            if ln == 512:
                                nc.scalar.activation(esb[:, eo:eo + 1024], et,
                                                     AF.Exp)
                            else:
                                ev = et.rearrange("p (b x) -> p b x", b=2)[:, :, 0:ln]
                                nc.scalar.activation(
                                    esb[:, eo:eo + 2 * ln]
                                    .rearrange("p (b x) -> p b x", b=2), ev, AF.Exp)
                            # denominator partial sums on DVE
                            if qt == 0 and p == 0:
                                nc.vector.tensor_copy(s_acc[:, 0:2 * ln],
                                                      esb[:, eo:eo + 2 * ln])
                            else:
                                nc.vector.tensor_add(s_acc[:, 0:2 * ln],
                                                     s_acc[:, 0:2 * ln],
                                                     esb[:, eo:eo + 2 * ln])
                        # dense apply runs for this quarter
                        for ct in range(CCH):
                            for gl in range(8):
                                g = qt * 8 + gl
                                nc.tensor.matmul(
                                    u[ct][:, 0:ln],
                                    lhsT=vT[:, g * C + ct * 128:
                                            g * C + (ct + 1) * 128],
                                    rhs=esb[:, gl * ln:(gl + 1) * ln],
                                    start=(g == 0), stop=(g == MG - 1))
                        # previous tile's combine, emitted one quarter late so
                        # this tile's energy matmuls precede it in PE priority
                        if qt == 0 and pend is not None:
                            pam_tail(*pend)
                    pend = (ti, s, ln, u, s_acc)
                pam_tail(*pend)

                # shifted copy so odd-offset conv taps read 4-byte-aligned
                for i in range(ICH):
                    nc.vector.tensor_copy(feats_sh[i][:, 0:FLAT + 1],
                                          feats[i][:, 1:FLAT + 2])

                # ---- phase 4+5: conv3x3 with fused BN stats, per-ot
                # AllReduce so chunk 0's BN/relu/store hides under chunk 1 ----
                inv_n = 1.0 / float(stat_count)
                for ot in range(CCH):
                    sparts = [lp.tile([128, len(CONV_TILES)], F32, tag="sparts",
                                      bufs=4, name=f"sparts{i}") for i in range(2)]
                    for ci, (s, ln) in enumerate(CONV_TILES):
                        yp = psp.tile([128, 512], F32, tag="u", bufs=2, name="yp")
                        first = True
                        for tap in range(9):
                            off = (tap // 3 - 1) * PADW + (tap % 3 - 1)
                            o = 1 + s + off
                            for ic in range(ICH):
                                src = feats[ic][:, o:o + ln] if o % 2 == 0 \
                                    else feats_sh[ic][:, o - 1:o - 1 + ln]
                                nc.tensor.matmul(
                                    yp[:, 0:ln],
                                    lhsT=fw[ic][:, (tap * 2 + ot) * 128:
                                                (tap * 2 + ot + 1) * 128],
                                    rhs=src,
                                    start=first, stop=(tap == 8 and ic == ICH - 1))
                                first = False
                        a = s - 66
                        nc.vector.tensor_copy(y_sb[ot][:, a:a + ln], yp[:, 0:ln])
                        # zero the junk pad cols inside this strip, then stats
                        yv = y_sb[ot].rearrange("p (r w) -> p r w", w=PADW)
                        for c in (0, 65):
                            r0 = max(0, (a - c + PADW - 1) // PADW)
                            r1 = min(32, (a + ln - 1 - c) // PADW + 1)
                            if r1 > r0:
                                nc.vector.memset(yv[:, r0:r1, c:c + 1], 0.0)
                        nc.vector.tensor_reduce(sparts[0][:, ci:ci + 1],
                                                y_sb[ot][:, a:a + ln],
                                                axis=mybir.AxisListType.X, op=ALU.add)
                        sq = lp.tile([128, 512], F32, tag="sq", bufs=2, name="sq")
                        nc.scalar.activation(sq[:, 0:ln], y_sb[ot][:, a:a + ln],
                                             AF.Square,
                                             accum_out=sparts[1][:, ci:ci + 1])
                    for i in range(2):
                        nc.vector.tensor_reduce(st_sb[:, 2 * ot + i:2 * ot + i + 1],
                                                sparts[i],
                                                axis=mybir.AxisListType.X, op=ALU.add)

                # ---- phase 5: one AllReduce, BN math, relu, store ----
                nc.sync.dma_start(out=st_loc.ap(), in_=st_sb)
                if n_cores > 1:
                    nc.gpsimd.collective_compute(
                        "AllReduce", ALU.add,
                        replica_groups=[list(range(n_cores))],
                        ins=[st_loc.ap()], outs=[st_glob.ap()])
                else:
                    nc.gpsimd.dma_start(out=st_glob.ap(), in_=st_loc.ap())
                nc.sync.dma_start(out=stg, in_=st_glob.ap())

                # BN math for both channel chunks at once ([128, 2] columns)
                mean = lp.tile([128, 2], F32, tag="bn_t", bufs=8, name="mean")
                msq = lp.tile([128, 2], F32, tag="bn_t", bufs=8, name="msq")
                var = lp.tile([128, 2], F32, tag="bn_t", bufs=8, name="var")
                m2 = lp.tile([128, 2], F32, tag="bn_t", bufs=8, name="m2")
                std = lp.tile([128, 2], F32, tag="bn_t", bufs=8, name="std")
                rstd = lp.tile([128, 2], F32, tag="bn_t", bufs=8, name="rstd")
                sc2 = lp.tile([128, 2], F32, tag="bn_t", bufs=8, name="sc2")
                bi2 = lp.tile([128, 2], F32, tag="bn_t", bufs=8, name="bi2")
                t0 = lp.tile([128, 2], F32, tag="bn_t", bufs=8, name="t0")
                stv = stg.rearrange("p (o i) -> p o i", i=2)
                nc.vector.tensor_scalar_mul(mean, stv[:, :, 0], inv_n)
                nc.vector.tensor_scalar_mul(msq, stv[:, :, 1], inv_n)
                nc.vector.tensor_mul(m2, mean, mean)
                nc.vector.tensor_sub(var, msq, m2)
                nc.scalar.activation(std, var, AF.Sqrt, bias=epsc)
                nc.vector.reciprocal(rstd, std)
                nc.vector.tensor_mul(sc2, bnsc, rstd)
                nc.vector.tensor_mul(t0, mean, sc2)
                nc.vector.tensor_sub(bi2, bnbi, t0)
                # y = relu(y*scale' + bias') on valid cols, then store
                for ot in range(CCH):
                    stage = lp.tile([128, 32 * W], F32, tag="stage", bufs=4,
                                    name="stage")
                    yvv = y_sb[ot].rearrange("p (r w) -> p r w", w=PADW)
                    for hf in range(2):
                        ysrc = yvv[:, hf * 16:hf * 16 + 16, 1:65]
                        nc.scalar.activation(
                            stage[:, hf * 1024:(hf + 1) * 1024]
                            .rearrange("p (r w) -> p r w", w=W), ysrc, AF.Relu,
                            bias=bi2[:, ot:ot + 1], scale=sc2[:, ot:ot + 1])
                        for dq in range(2):
                            o = hf * 1024 + dq * 512
                            nc.sync.dma_start(
                                out=y_out.ap()[ot][:, o:o + 512],
                                in_=stage[:, o:o + 512])

    nc.compile()
    return nc


_CACHE = {}


def _get_nc(n_cores=8, stat_count=4 * N):
    key = (n_cores, stat_count)
    if key not in _CACHE:
        _CACHE[key] = build(n_cores, stat_count)
    return _CACHE[key]


def make_in_maps(x, q_w, q_b, k_w, k_b, v_w, v_b, gamma_p, gamma_c,
                 fuse_w, bn_scale, bn_bias, cores=8):
    f4 = np.float32
    shared = {}
    # 4 partition-replicas of the q/k projection weights and biases
    qwT = np.asarray(q_w, f4).T                  # [C, RC]
    kwT = np.asarray(k_w, f4).T
    shared["qwT4"] = np.ascontiguousarray(np.tile(qwT, (1, 4))) \
        .astype(ml_dtypes.bfloat16)              # [C, 128]
    shared["kwT4"] = np.ascontiguousarray(np.tile(kwT, (1, 4))) \
        .astype(ml_dtypes.bfloat16)
    shared["qb4"] = np.tile(np.asarray(q_b, f4), 4).reshape(128, 1)
    shared["kb4"] = np.tile(np.asarray(k_b, f4), 4).reshape(128, 1)
    shared["vwT"] = np.ascontiguousarray(np.asarray(v_w, f4).T).astype(ml_dtypes.bfloat16)
    # fuse_w [256, 512, 3, 3] -> [ic, i, tap*2*128 + ot*128 + o] in bf16
    fwr = np.asarray(fuse_w, f4).reshape(CCH, 128, ICH, 128, 3, 3)
    fwt = np.ascontiguousarray(fwr.transpose(2, 3, 4, 5, 0, 1))  # ic,i,kh,kw,ot,o
    shared["fw"] = fwt.reshape(ICH, 128, 9 * 2 * 128).astype(ml_dtypes.bfloat16)
    gp = np.asarray(gamma_p, f4).ravel()[0]
    gcv = np.asarray(gamma_c, f4).ravel()[0]
    with np.errstate(divide="ignore"):
        gpi = np.float32(1.0) / gp
    shared["gpinv"] = np.full((128, 1), gpi, f4).astype(ml_dtypes.bfloat16)
    shared["gpvb"] = np.ascontiguousarray(
        (gp * np.asarray(v_b, f4)).reshape(CCH, 128).T)
    shared["gc"] = np.full((128, 1), gcv, f4)
    shared["bnsc"] = np.ascontiguousarray(np.asarray(bn_scale, f4).reshape(CCH, 128).T)
    shared["bnbi"] = np.ascontiguousarray(np.asarray(bn_bias, f4).reshape(CCH, 128).T)
    shared["ones_m1b"] = np.ones((1, 128), ml_dtypes.bfloat16)
    shared["ident"] = np.eye(128, dtype=ml_dtypes.bfloat16)

    xs = np.asarray(x, f4).reshape(B, C, N).astype(ml_dtypes.bfloat16)
    zrow = np.zeros((C, W), ml_dtypes.bfloat16)
    in_maps = []
    for core in range(cores):
        b, j = core // 2, core % 2
        xf = np.ascontiguousarray(xs[b])
        if j == 0:
            band = np.concatenate([zrow, xf[:, 0:33 * W]], axis=1)
            mfv, mlv = 0.0, 1.0
        else:
            band = np.concatenate([xf[:, 31 * W:], zrow], axis=1)
            mfv, mlv = 1.0, 0.0
        m = dict(shared)
        m["x_full"] = xf
        m["x_band"] = np.ascontiguousarray(band)
        m["mf"] = np.full((128, 1), mfv, f4)
        m["ml"] = np.full((128, 1), mlv, f4)
        in_maps.append(m)
    return in_maps


def kernel(**inputs):
    nc = _get_nc(8)
    in_maps = make_in_maps(**inputs)
    r = run_bass_kernel_spmd(nc, in_maps, core_ids=list(range(8)))
    out = np.empty((B, C, H, W), np.float32)
    for core in range(8):
        b, j = core // 2, core % 2
        y2 = r.results[core]["y_out"]  # [2, 128, 2048]
        out[b, 0:128, 32 * j:32 * j + 32, :] = y2[0].reshape(128, 32, W)
        out[b, 128:256, 32 * j:32 * j + 32, :] = y2[1].reshape(128, 32, W)
    return out
